# revision 1
# baseline (speedup 1.0000x reference)
"""DealerGraphSAGE (2-layer SAGEConv, mean aggregation) on 8 Trainium2 NeuronCores.

Self-contained kernel: takes full inputs, shards internally, returns full output.

Design: nodes are range-partitioned across 8 cores (12500 each) and degree-sorted
into a rank space of 12544 rows per core (98 windows x 128 partitions). Both
feature tables live in DRAM in rank-major layout with 256B row stride:
x_t[core*12544+rank] holds [features(9), 1.0, pad...] and t_ext (the AllGathered
layer-1 outputs) holds [t(32), junk...]. Because both tables share the rank
geometry, one int16 CSR slot grid serves both layers: the 100352-row table is
split into 4 shards of 25088 rows (int16 index limit), edges are bucketed into
(window, shard) slot runs padded to the max count over partitions and cores
(pads point at an all-zero row), and each shard's columns are gathered with
batched GPSIMD dma_gather calls (64B payloads for layer 1, 128B for layer 2,
256B stride) spread over 4 SWDGE queues. DVE segment-reduces per (window,shard)
run accumulate into a per-shard partial tensor, a final strided reduce folds
shards, and the dense SAGE algebra runs on the PE exactly as in the reference:
hT = relu(W1_l^T mean^T + W1_r^T x^T + b1), t = h @ W2_l (AllGathered),
z = agg2/deg + h @ W2_r + b2. The host unpermutes z back to natural order.
"""
import numpy as np
from contextlib import ExitStack

import concourse.bacc as bacc
import concourse.tile as tile
import concourse.bass as bass
import concourse.mybir as mybir
from concourse.masks import make_identity
from concourse.library_config import mlp
from concourse._compat import exact_div

N_NODES = 100000
N_CORES = 8
LOCAL = N_NODES // N_CORES          # 12500
WINP = 128
NWIN = (LOCAL + WINP - 1) // WINP   # 98
LOCAL_PAD = NWIN * WINP             # 12544
NSHARD = 4
SH = 2 * LOCAL_PAD                  # 25088 rows per shard (2 core blocks)
TROWS = N_CORES * LOCAL_PAD         # 100352
F_IN, H, OUT = 9, 64, 32
ESTEP = 64                          # table row stride in f32 (256B)
E1 = 16                             # layer-1 gather payload (64B; cols 0:10 used)
E2 = 32                             # layer-2 gather payload (128B)
ZROW = LOCAL                        # per-shard all-zero row (local index 12500)
CHUNK_C = 64                        # max slot columns per dma_gather call
NQ = 4                              # SWDGE queues
MMCHUNK = 512


def dma_gather_small(gp, out_ap, in_ap, idxs_ap, num_idxs, elem_size, elem_step,
                     queue_num=0):
    """gpsimd.dma_gather minus the elem_size%256 assert (elem_step carries the
    256B-multiple stride). Payload = elem_size elements from each indexed row."""
    from concourse.bass import MemorySpace
    from concourse import ap_utils
    assert idxs_ap.dtype == mybir.dt.int16
    assert in_ap.space == MemorySpace.DRAM and out_ap.space == MemorySpace.SBUF
    assert ap_utils.ap_is_contiguous(out_ap.ap[1:])
    assert ap_utils.ap_is_contiguous(idxs_ap.ap[1:])
    assert in_ap.ap[0][0] == elem_step
    stride_bytes_256 = exact_div(elem_step * mybir.dt.size(in_ap.dtype), 256)
    _in_ap = gp.lower_ap_dma(in_ap, for_custom_bir_dma=True)
    return gp.add_instruction(
        mybir.InstDMAGatherAnt(
            name=gp.bass.get_next_instruction_name(),
            ins=[*_in_ap, gp.lower_ap(idxs_ap),
                 gp.lower_val_access(gp.to_reg(num_idxs))],
            outs=[gp.lower_ap(out_ap)],
            transpose=False, num_idxs=num_idxs, elem_size=elem_size,
            stride_bytes_256=stride_bytes_256, gen_mode=0, single_packet=False,
            queue_num=queue_num, sbuf_tokens_per_rank=0,
            sbuf_free_dim_per_rank=0, sbuf_free_dim_pad_per_rank=0,
            sbuf_byte_offset=0,
        )
    )


def host_prep(x, edge_index):
    src = np.asarray(edge_index[0], dtype=np.int64)
    dst = np.asarray(edge_index[1], dtype=np.int64)
    x = np.asarray(x, dtype=np.float32)

    core_of = dst // LOCAL
    orders = []
    rank_of_all = np.empty(N_NODES, dtype=np.int64)
    for c in range(N_CORES):
        sel = np.nonzero(core_of == c)[0]
        d = dst[sel] - c * LOCAL
        deg = np.bincount(d, minlength=LOCAL)
        order = np.argsort(-deg, kind="stable")       # rank -> local id
        rank_of = np.empty(LOCAL, dtype=np.int64)
        rank_of[order] = np.arange(LOCAL)
        rank_of_all[c * LOCAL:(c + 1) * LOCAL] = rank_of
        orders.append(order)

    # rank-major padded feature table (shared by all cores)
    x_t = np.zeros((TROWS, ESTEP), dtype=np.float32)
    rows_t = (np.arange(N_NODES) // LOCAL) * LOCAL_PAD + rank_of_all
    x_t[rows_t, 0:F_IN] = x
    x_t[rows_t, F_IN] = 1.0

    trow_of_src = (src // LOCAL) * LOCAL_PAD + rank_of_all[src]

    # per-core edge cells: (window, shard, partition) counts -> shared K grid
    per_core = []
    K = np.zeros((NWIN, NSHARD), dtype=np.int64)
    for c in range(N_CORES):
        sel = np.nonzero(core_of == c)[0]
        r = rank_of_all[dst[sel]]                    # dst rank 0..12499
        w = r // WINP
        p = r % WINP
        tr = trow_of_src[sel]
        sh = tr // SH
        loc = tr % SH
        cnt = np.bincount((w * WINP + p) * NSHARD + sh,
                          minlength=LOCAL_PAD * NSHARD
                          ).reshape(NWIN, WINP, NSHARD)
        np.maximum(K, cnt.max(axis=1), out=K)
        per_core.append({"w": w, "p": p, "sh": sh, "loc": loc})
    np.maximum(K, 1, out=K)

    # column layout: shard-major, window-minor
    CB = np.zeros(NSHARD + 1, dtype=np.int64)       # shard column base
    np.cumsum(K.sum(axis=0), out=CB[1:])
    COLS = int(CB[-1])
    WB = np.zeros((NSHARD, NWIN), dtype=np.int64)   # (shard, window) run base
    for s in range(NSHARD):
        WB[s, 0] = CB[s]
        np.cumsum(K[:-1, s], out=WB[s, 1:])
        WB[s, 1:] += CB[s]

    for c in range(N_CORES):
        pc = per_core[c]
        w, p, sh, loc = pc["w"], pc["p"], pc["sh"], pc["loc"]
        key = ((sh * NWIN + w) * WINP + p)
        eo = np.argsort(key, kind="stable")
        ks = key[eo]
        # sequence number within each (sh, w, p) cell
        starts = np.r_[0, np.nonzero(np.diff(ks))[0] + 1]
        seq = np.arange(len(ks)) - np.repeat(starts, np.diff(np.r_[starts, len(ks)]))
        col = WB[sh[eo], w[eo]] + seq
        csr = np.full((WINP, COLS), ZROW, dtype=np.int16)
        csr[p[eo], col] = loc[eo].astype(np.int16)
        # wrap for dma_gather: list position i=(c*128+p) at idx[i%16, i//16],
        # replicated across the 8 gpsimd 16-partition groups
        tile16 = csr.reshape(8, 16, COLS).transpose(1, 2, 0).reshape(16, COLS * 8)
        pc["idx"] = np.tile(tile16, (8, 1))
        pc["x_self"] = x_t[c * LOCAL_PAD:(c + 1) * LOCAL_PAD, 0:E1].copy()
        pc["order"] = orders[c]
        del pc["w"], pc["p"], pc["sh"], pc["loc"]

    # gather call chunks: per shard, window runs packed to <= CHUNK_C columns
    chunks = []                                     # (shard, c0, cn, [(w, off, kw)...])
    for s in range(NSHARD):
        c0 = int(CB[s])
        runs = []
        for w in range(NWIN):
            kw = int(K[w, s])
            if sum(k for _, _, k in runs) + kw > CHUNK_C:
                chunks.append((s, c0, sum(k for _, _, k in runs), runs))
                c0 += sum(k for _, _, k in runs)
                runs = []
            runs.append((w, c0 + sum(k for _, _, k in runs) - c0, kw))
        if runs:
            chunks.append((s, c0, sum(k for _, _, k in runs), runs))
    # fix offsets: off = run base within chunk
    fixed = []
    for s, c0, cn, runs in chunks:
        off = 0
        rr = []
        for w, _, kw in runs:
            rr.append((w, off, kw))
            off += kw
        fixed.append((s, c0, cn, rr))

    return {"x_t": x_t, "K": K, "COLS": COLS, "chunks": fixed}, per_core


def build_program(K, COLS, chunks):
    import os
    DIAG = os.environ.get("KDIAG", "")          # comma list: nol1g,nol2g,nored,nocoll
    diag = set(DIAG.split(",")) if DIAG else set()
    AMP = int(os.environ.get("KAMP", "1"))      # timing amplification (diagnostics)
    dt = mybir.dt
    f32 = dt.float32
    AX = mybir.AxisListType
    OPS = mybir.AluOpType
    ACT = mybir.ActivationFunctionType

    nc = bacc.Bacc("TRN2", target_bir_lowering=False, debug=False,
                   num_devices=N_CORES, num_swdge_queues=NQ)

    x_t = nc.dram_tensor("x_t", [TROWS, ESTEP], f32, kind="ExternalInput").ap()
    idx = nc.dram_tensor("idx", [128, COLS * 8], dt.int16, kind="ExternalInput").ap()
    x_self = nc.dram_tensor("x_self", [LOCAL_PAD, E1], f32, kind="ExternalInput").ap()
    w1l = nc.dram_tensor("w1l", [F_IN, H], f32, kind="ExternalInput").ap()
    w1r = nc.dram_tensor("w1r", [F_IN, H], f32, kind="ExternalInput").ap()
    b1 = nc.dram_tensor("b1", [H], f32, kind="ExternalInput").ap()
    w2l = nc.dram_tensor("w2l", [H, OUT], f32, kind="ExternalInput").ap()
    w2rb = nc.dram_tensor("w2rb", [H + 1, OUT], f32, kind="ExternalInput").ap()
    z_out = nc.dram_tensor("z_out", [LOCAL_PAD, OUT], f32, kind="ExternalOutput").ap()
    t_local = nc.dram_tensor("t_local", [LOCAL_PAD, ESTEP], f32).ap()
    t_ext = nc.dram_tensor("t_ext", [TROWS, ESTEP], f32).ap()

    with tile.TileContext(nc) as tc, ExitStack() as ctx:
        pool = ctx.enter_context(tc.tile_pool(name="main", bufs=1))
        gpool = ctx.enter_context(tc.tile_pool(name="gather", bufs=3))
        ipool = ctx.enter_context(tc.tile_pool(name="ix", bufs=3))
        pp_t = ctx.enter_context(tc.tile_pool(name="pp_t", bufs=2, space="PSUM"))
        pp_h = ctx.enter_context(tc.tile_pool(name="pp_h", bufs=2, space="PSUM"))
        pp_w = ctx.enter_context(tc.tile_pool(name="pp_w", bufs=1, space="PSUM"))

        nc.gpsimd.load_library(mlp)

        ident = pool.tile([128, 128], f32)
        make_identity(nc, ident[:])
        w1l_sb = pool.tile([F_IN, H], f32)
        nc.sync.dma_start(out=w1l_sb[:], in_=w1l[:])
        w1r_sb = pool.tile([F_IN, H], f32)
        nc.sync.dma_start(out=w1r_sb[:], in_=w1r[:])
        b1_sb = pool.tile([H, 1], f32)
        nc.sync.dma_start(out=b1_sb[:], in_=b1[:, None])
        w2l_sb = pool.tile([H, OUT], f32)
        nc.sync.dma_start(out=w2l_sb[:], in_=w2l[:])
        w2rb_sb = pool.tile([H + 1, OUT], f32)
        nc.sync.dma_start(out=w2rb_sb[:], in_=w2rb[:])
        zrow = pool.tile([1, OUT], f32)
        nc.vector.memset(zrow[:], 0.0)

        hT = pool.tile([H + 1, LOCAL_PAD], f32)
        recip = pool.tile([WINP, NWIN, 1], f32)

        def gather_phase(table, elem, dred, aggS, tag):
            if ("no" + tag + "g") in diag:
                nc.vector.memset(aggS[:], 0.0)
                return
            qi = 0
            for s, c0, cn, runs in chunks:
                ixt = ipool.tile([128, CHUNK_C * 8], dt.int16, tag="ix" + tag)
                nc.sync.dma_start(out=ixt[:, 0:cn * 8], in_=idx[:, c0 * 8:(c0 + cn) * 8])
                g = gpool.tile([128, CHUNK_C, elem], f32, tag="g" + tag)
                dma_gather_small(nc.gpsimd, g[:, 0:cn, :],
                                 table[s * SH:(s + 1) * SH, :], ixt[:, 0:cn * 8],
                                 cn * 128, elem, ESTEP, queue_num=qi % NQ)
                qi += 1
                if "nored" in diag:
                    continue
                for w, off, kw in runs:
                    nc.vector.tensor_reduce(
                        out=aggS[:, s, w, :],
                        in_=g[:, off:off + kw, 0:dred].rearrange("p j d -> p d j"),
                        axis=AX.X, op=OPS.add)
            if "nored" in diag:
                nc.vector.memset(aggS[:], 0.0)

        # ---------------- layer 1 ----------------
        with tc.tile_pool(name="l1", bufs=1) as p1:
          def body_l1(_i=None):
              aggS1 = p1.tile([WINP, NSHARD, NWIN, F_IN + 1], f32, tag="aggS1")
              gather_phase(x_t, E1, F_IN + 1, aggS1, "l1")
              agg1 = p1.tile([WINP, NWIN, F_IN + 1], f32)
              nc.vector.tensor_reduce(
                  out=agg1[:], in_=aggS1[:].rearrange("p s w d -> p w d s"),
                  axis=AX.X, op=OPS.add)

              xs = p1.tile([WINP, NWIN, E1], f32)
              nc.sync.dma_start(
                  out=xs[:],
                  in_=x_self[:].rearrange("(w p) e -> p w e", p=WINP))

              nc.vector.tensor_scalar(out=recip[:], in0=agg1[:, :, F_IN:F_IN + 1],
                                      scalar1=1.0, scalar2=None, op0=OPS.max)
              nc.vector.reciprocal(out=recip[:], in_=recip[:])
              meanS = p1.tile([WINP, NWIN, F_IN], f32)
              nc.vector.tensor_tensor(out=meanS[:], in0=agg1[:, :, 0:F_IN],
                                      in1=recip[:].to_broadcast([WINP, NWIN, F_IN]),
                                      op=OPS.mult)

              # hT = relu(W1_l^T mean^T + W1_r^T x^T + b1); row H = ones
              nc.vector.memset(hT[H:H + 1, :], 1.0)
              WPC = MMCHUNK // WINP
              n_chunks = (NWIN + WPC - 1) // WPC
              for ci in range(n_chunks):
                  w0 = ci * WPC
                  wn = min(WPC, NWIN - w0)
                  cols = wn * WINP
                  mT = p1.tile([F_IN, MMCHUNK], f32, tag="mT")
                  xT = p1.tile([F_IN, MMCHUNK], f32, tag="xT")
                  for wi in range(wn):
                      ps_m = pp_t.tile([F_IN, WINP], f32, tag="tp9")
                      nc.tensor.transpose(out=ps_m[:], in_=meanS[:, w0 + wi, :], identity=ident[:])
                      nc.scalar.activation(out=mT[:, wi * WINP:(wi + 1) * WINP], in_=ps_m[:], func=ACT.Copy)
                      ps_x = pp_t.tile([F_IN, WINP], f32, tag="tp9")
                      nc.tensor.transpose(out=ps_x[:], in_=xs[:, w0 + wi, 0:F_IN], identity=ident[:])
                      nc.scalar.activation(out=xT[:, wi * WINP:(wi + 1) * WINP], in_=ps_x[:], func=ACT.Copy)
                  ps_h = pp_h.tile([H, MMCHUNK], f32, tag="ph")
                  nc.tensor.matmul(out=ps_h[:, 0:cols], lhsT=w1l_sb[:], rhs=mT[:, 0:cols], start=True, stop=False)
                  nc.tensor.matmul(out=ps_h[:, 0:cols], lhsT=w1r_sb[:], rhs=xT[:, 0:cols], start=False, stop=True)
                  nc.scalar.activation(out=hT[0:H, w0 * WINP:w0 * WINP + cols], in_=ps_h[:, 0:cols],
                                       func=ACT.Relu, bias=b1_sb[:])

              # t = h @ W2_l in rank order -> t_local (cols 0:OUT)
              tNP = p1.tile([WINP, NWIN, OUT], f32)
              for ci in range(n_chunks):
                  w0 = ci * WPC
                  wn = min(WPC, NWIN - w0)
                  cols = wn * WINP
                  ps_tc = pp_h.tile([OUT, MMCHUNK], f32, tag="pt")
                  nc.tensor.matmul(out=ps_tc[:, 0:cols], lhsT=w2l_sb[:],
                                   rhs=hT[0:H, w0 * WINP:w0 * WINP + cols], start=True, stop=True)
                  tTc = p1.tile([OUT, MMCHUNK], f32, tag="tTc")
                  nc.scalar.activation(out=tTc[:, 0:cols], in_=ps_tc[:, 0:cols], func=ACT.Copy)
                  for wi in range(wn):
                      ps_w = pp_w.tile([WINP, OUT], f32, tag="tw")
                      nc.tensor.transpose(out=ps_w[:], in_=tTc[:, wi * WINP:(wi + 1) * WINP],
                                          identity=ident[0:OUT, 0:OUT])
                      nc.vector.tensor_copy(out=tNP[:, w0 + wi, :], in_=ps_w[:])
              nc.sync.dma_start(
                  out=t_local[:].rearrange("(w p) e -> p w e", p=WINP)[:, :, 0:OUT],
                  in_=tNP[:])
              nc.sync.dma_start(out=t_local[ZROW:ZROW + 1, 0:OUT], in_=zrow[:])
          if AMP == 1:
              body_l1()
          else:
              with tc.For_i(0, AMP, 1) as _i:
                  body_l1(_i)

        if "nocoll" not in diag:
            nc.gpsimd.collective_compute(
                "AllGather", mybir.AluOpType.bypass,
                replica_groups=[list(range(N_CORES))],
                ins=[t_local[:]], outs=[t_ext[:]])

        # ---------------- layer 2 ----------------
        with tc.tile_pool(name="l2", bufs=1) as p2:
          def body_l2(_i=None):
              aggS2 = p2.tile([WINP, NSHARD, NWIN, OUT], f32, tag="aggS2")
              gather_phase(t_ext, E2, OUT, aggS2, "l2")
              agg2 = p2.tile([WINP, NWIN, OUT], f32)
              nc.vector.tensor_reduce(
                  out=agg2[:], in_=aggS2[:].rearrange("p s w d -> p w d s"),
                  axis=AX.X, op=OPS.add)

              zNP = p2.tile([WINP, NWIN, OUT], f32)
              for w in range(NWIN):
                  ps_zr = pp_w.tile([WINP, OUT], f32, tag="zw")
                  nc.tensor.matmul(out=ps_zr[:], lhsT=hT[:, w * WINP:(w + 1) * WINP],
                                   rhs=w2rb_sb[:], start=True, stop=True)
                  nc.vector.tensor_tensor(out=zNP[:, w, :], in0=agg2[:, w, :],
                                          in1=recip[:, w, :].to_broadcast([WINP, OUT]),
                                          op=OPS.mult)
                  nc.vector.tensor_tensor(out=zNP[:, w, :], in0=zNP[:, w, :],
                                          in1=ps_zr[:], op=OPS.add)
              nc.sync.dma_start(
                  out=z_out[:].rearrange("(w p) e -> p w e", p=WINP),
                  in_=zNP[:])
          if AMP == 1:
              body_l2()
          else:
              with tc.For_i(0, AMP, 1) as _i:
                  body_l2(_i)

    nc.compile()
    return nc


_CACHE = {}


def _get_program(x, edge_index):
    shared, per_core = host_prep(x, edge_index)
    import os
    key = (shared["COLS"], shared["K"].tobytes(), os.environ.get("KDIAG", ""), os.environ.get("KAMP", "1"))
    if key not in _CACHE:
        _CACHE[key] = build_program(shared["K"], shared["COLS"], shared["chunks"])
    return _CACHE[key], shared, per_core


def make_in_maps(shared, per_core, W1_l, b1, W1_r, W2_l, b2, W2_r):
    w2rb = np.concatenate([np.asarray(W2_r, np.float32),
                           np.asarray(b2, np.float32)[None, :]], axis=0)
    in_maps = []
    for c in range(N_CORES):
        pc = per_core[c]
        in_maps.append({
            "x_t": shared["x_t"], "idx": pc["idx"], "x_self": pc["x_self"],
            "w1l": np.asarray(W1_l, np.float32), "w1r": np.asarray(W1_r, np.float32),
            "b1": np.asarray(b1, np.float32),
            "w2l": np.asarray(W2_l, np.float32), "w2rb": w2rb,
        })
    return in_maps


def assemble_output(outs, per_core):
    z = np.empty((N_NODES, OUT), dtype=np.float32)
    for c in range(N_CORES):
        zc = outs[c]["z_out"][:LOCAL]
        z[c * LOCAL + per_core[c]["order"]] = zc
    return z


def kernel(x, edge_index, W1_l, b1, W1_r, W2_l, b2, W2_r):
    nc, shared, per_core = _get_program(x, edge_index)
    in_maps = make_in_maps(shared, per_core, W1_l, b1, W1_r, W2_l, b2, W2_r)
    from concourse.bass_utils import run_bass_kernel_spmd
    res = run_bass_kernel_spmd(nc, in_maps, list(range(N_CORES)))
    return assemble_output(res.results, per_core)

